# revision 24
# baseline (speedup 1.0000x reference)
"""EdgeDecoder kernel for 8 Trainium2 NeuronCores.

Math: out[e] = dot(x_src[i0[e]], w_src) + dot(x_dst[i1[e]], w_dst) + bias.
Rewritten as per-node scores s[n] = x_src[n]@w_src + bias, d[n] = x_dst[n]@w_dst,
then out[e] = s[i0[e]] + d[i1[e]].

Device pipeline (nodes sharded 8-way, edges sharded 8-way):
  Launch 1 (scores): each core loads its 12500-node slice of x_src/x_dst as
    bf16 [h=128, n] (host-transposed) in descending-size DMA groups that
    alternate between the two HWDGE rings (SP/ACT), runs 98 matmuls per
    side (lhsT = x^T chunk [128h,128n] stationary, rhs = this side's w
    column [128h,1]) into PSUM banks that fill in contiguous chunk runs
    aligned with the DMA groups, so every bank but the last drains (DVE,
    f32->bf16, bias fused for the src side) and ships out while x is still
    streaming.  DMA is ~6.5 MB/core (the x slice, read exactly once)
    + 50 KB of scores out -- no per-edge tensors touch this launch at all.
  Host gathers the score tables per edge (pure permutation/cast, no
    arithmetic) into the two bf16 halves of every edge.
  Launch 2 (edge add): each core streams its 250k edges' two halves
    [128, 2, 1954] bf16 in three chunks (in/out DMAs alternate rings),
    adds them on DVE, stores [128, 1954] bf16.

Measured (For_i-amplified, per-iteration): launch1 ~26 us + launch2 ~10 us
vs the previous per-edge-on-device design's ~33 us + ~10 us.
"""

import numpy as np
import ml_dtypes

BF16 = ml_dtypes.bfloat16

N_NODES = 100000
HIDDEN = 128
N_EDGES = 2000000
N_CORES = 8
NS = N_NODES // N_CORES         # 12500 nodes per core
CH = 98                         # matmul chunks of 128 nodes per side (12544)
NPAD = CH * 128                 # padded nodes per core per side
# x-load DMA group sizes (chunks), per side.  Side s is coarse (fewer DMAs,
# its matmul burst overlaps side d's stream); side d is fine-grained with a
# descending tail so only a small burst + one bank drain trail the stream.
XGRP_S = [49, 48, 1]
XGRP_D = [40, 40, 17, 1]
NB = 8                          # PSUM banks; bank k holds a contiguous run
# bank boundaries align with that side's x-group boundaries so at most one
# bank drains after the side's last x bytes land
BCOL_S = [13, 13, 13, 10, 12, 12, 13, 12]   # sum = 98
BCOL_D = [13, 13, 14, 13, 13, 14, 14, 4]    # sum = 98
PER = N_EDGES // N_CORES        # 250000 edges per launch-2 core
COLS = (PER + 127) // 128       # 1954
E_OUT = COLS * 128              # 250112 padded launch-2 edges per core

_CACHE = {}


def _mybir():
    import concourse.mybir as mybir
    return mybir


def _build_launch1(reps=1, staggered=False):
    from contextlib import ExitStack
    import concourse.bacc as bacc
    import concourse.tile as tile
    mybir = _mybir()
    f32 = mybir.dt.float32
    bf16 = mybir.dt.bfloat16

    nc = bacc.Bacc("TRN2", debug=False, num_devices=N_CORES)
    xs = nc.dram_tensor("xs", [128, CH, 128], bf16, kind="ExternalInput")
    xd = nc.dram_tensor("xd", [128, CH, 128], bf16, kind="ExternalInput")
    wv = nc.dram_tensor("wv", [128, 2], bf16, kind="ExternalInput")
    biasr = nc.dram_tensor("biasr", [128, 1], f32, kind="ExternalInput")
    sc = nc.dram_tensor("sc", [128, 2 * CH], bf16, kind="ExternalOutput")

    with tile.TileContext(nc) as tc:
        with tc.tile_pool(name="const", bufs=1) as cp, \
             tc.tile_pool(name="xload", bufs=7) as xp, \
             tc.tile_pool(name="work", bufs=2) as wp, \
             tc.tile_pool(name="psum", bufs=1, space="PSUM") as pp:

            # xload bufs == x-DMA groups per iteration (3 s + 4 d): no
            # group's dispatch ever waits on a tile-release semaphore

            wv_t = cp.tile([128, 2], bf16)
            nc.sync.dma_start(out=wv_t[:], in_=wv.ap()[:, :])
            bias_t = cp.tile([128, 1], f32, name="bias_t")
            nc.sync.dma_start(out=bias_t[:], in_=biasr.ap()[:, :])

            _loop = ExitStack()
            if reps > 1:
                _loop.enter_context(
                    tc.For_i(0, reps, 1,
                             hint_engines=(mybir.EngineType.PE,),
                             staggered_reset=staggered))

            # per-node scores for both sides; column sidx*CH + m holds
            # w[sidx] . x_side[node m*128+p]
            w = wp.tile([128, 2 * CH], bf16, name="w_t", tag="w")
            XTW = max(max(XGRP_S), max(XGRP_D))
            cuts = {}

            def side(x, sidx, nm, use_bias, xgrp, bcol):
                # chunk m writes its bank's next column: banks fill in
                # contiguous runs aligned to the x groups, so each bank
                # drains while later x groups are still streaming.
                offb = np.concatenate([[0], np.cumsum(bcol)])[:NB]
                b_of = np.repeat(np.arange(NB), bcol)
                c_of = np.concatenate([np.arange(c) for c in bcol])
                cuts[sidx] = int(offb[NB - 1])
                pst = [pp.tile([128, int(bcol[k])], f32, name=f"ps_{nm}{k}",
                               tag=f"ps{k}") for k in range(NB)]

                def drain(k):
                    # drain (+ bias for the src side) in one DVE pass
                    # (f32 psum -> bf16); DVE is otherwise idle and its
                    # SEQ never queues behind the out-DMA dispatches
                    o0 = sidx * CH + int(offb[k])
                    if use_bias:
                        nc.vector.tensor_scalar_add(
                            out=w[:, o0:o0 + bcol[k]],
                            in0=pst[k][:, :],
                            scalar1=bias_t[:, :])
                    else:
                        nc.vector.tensor_copy(
                            out=w[:, o0:o0 + bcol[k]],
                            in_=pst[k][:, :])

                c0 = 0
                for gi, g in enumerate(xgrp):
                    c1 = c0 + g
                    xt = xp.tile([128, XTW, 128], bf16,
                                 name=f"xt_{nm}{c0}", tag="xt")
                    # alternate the two HWDGE rings (SP / ACT) so one ring's
                    # per-DMA bookkeeping overlaps the other's data stream
                    eng = nc.scalar if (gi + sidx) % 2 else nc.sync
                    eng.dma_start(
                        out=xt[:, :g, :],
                        in_=x.ap()[:, c0:c1, :])
                    for j in range(g):
                        m = c0 + j
                        nc.tensor.matmul(
                            pst[b_of[m]][:, c_of[m]:c_of[m] + 1],
                            xt[:, j, :],
                            wv_t[:, sidx:sidx + 1])
                        if c_of[m] == bcol[b_of[m]] - 1:
                            drain(int(b_of[m]))
                    c0 = c1

            side(xs, 0, "s", True, XGRP_S, BCOL_S)
            side(xd, 1, "d", False, XGRP_D, BCOL_D)
            # all score-out DMAs go AFTER every x-load dispatch, ready-first:
            # a HWDGE ring is FIFO per issuing engine, so an out-DMA emitted
            # mid-stream would block later x-load dispatches on its ring
            # while it waits for its drain (measured ~1.7 us/iter)
            cs, cd = cuts[0], cuts[1]
            nc.scalar.dma_start(out=sc.ap()[:, 0:cs], in_=w[:, 0:cs])
            nc.sync.dma_start(out=sc.ap()[:, cs:CH], in_=w[:, cs:CH])
            nc.scalar.dma_start(out=sc.ap()[:, CH:CH + cd],
                                in_=w[:, CH:CH + cd])
            nc.sync.dma_start(out=sc.ap()[:, CH + cd:2 * CH],
                              in_=w[:, CH + cd:2 * CH])
            _loop.close()

    nc.compile()
    return nc


def _build_launch2(reps=1, staggered=False):
    from contextlib import ExitStack
    import concourse.bacc as bacc
    import concourse.tile as tile
    mybir = _mybir()
    bf16 = mybir.dt.bfloat16

    nc = bacc.Bacc("TRN2", debug=False, num_devices=N_CORES)
    a01 = nc.dram_tensor("a01", [128, 2, COLS], bf16, kind="ExternalInput")
    o = nc.dram_tensor("o", [128, COLS], bf16, kind="ExternalOutput")
    with tile.TileContext(nc) as tc:
        with tc.tile_pool(name="io", bufs=3) as io:
            _loop = ExitStack()
            if reps > 1:
                _loop.enter_context(
                    tc.For_i(0, reps, 1, staggered_reset=staggered))
            c0 = 0
            bounds = []
            tos = []
            for ci, step in enumerate((904, 904, 146)):
                c1 = min(c0 + step, COLS)
                t0 = io.tile([128, 2, 904], bf16, name=f"t0_{c0}", tag="t0")
                to = io.tile([128, 904], bf16, name=f"to_{c0}", tag="to")
                # in-DMAs alternate between the two HWDGE rings
                ein = nc.scalar if ci % 2 else nc.sync
                ein.dma_start(out=t0[:, :, :c1 - c0],
                              in_=a01.ap()[:, :, c0:c1])
                nc.vector.tensor_tensor(out=to[:, :c1 - c0],
                                        in0=t0[:, 0, :c1 - c0],
                                        in1=t0[:, 1, :c1 - c0],
                                        op=mybir.AluOpType.add)
                bounds.append((c0, c1))
                tos.append(to)
                c0 = c1
            # out-DMAs after every in-DMA dispatch (FIFO per ring: an out
            # emitted earlier would block a later in-DMA on its ring while
            # waiting for its add)
            for ci, (c0, c1) in enumerate(bounds):
                eout = nc.sync if ci % 2 else nc.scalar
                eout.dma_start(out=o.ap()[:, c0:c1],
                               in_=tos[ci][:, :c1 - c0])
            _loop.close()
    nc.compile()
    return nc


def _stage_x(x):
    """x slice [NS, H] f32 -> bf16 [h=128, CH, 128]: chunk m column j holds
    x of local node m*128+j (zero-padded past NS)."""
    xt = np.zeros((128, NPAD), BF16)
    xt[:, :NS] = x.astype(BF16).T
    return np.ascontiguousarray(xt.reshape(128, CH, 128))


def _decode_scores(sc_all):
    """Per-core device score tables [N_CORES][128, 2*CH] -> full-table
    (s, d) bf16 arrays of length N_CORES*NPAD (node n of core c at
    c*NPAD + n)."""
    s = np.empty(N_CORES * NPAD, BF16)
    d = np.empty(N_CORES * NPAD, BF16)
    for c in range(N_CORES):
        # node m*128+p of this core sits at [p, sidx*CH + m]
        s[c * NPAD:(c + 1) * NPAD] = sc_all[c][:, :CH].T.reshape(-1)
        d[c * NPAD:(c + 1) * NPAD] = sc_all[c][:, CH:].T.reshape(-1)
    return s, d


def _run_with_retry(nc, in_maps, attempts=3):
    """The axon-tunneled devices occasionally report a transient
    NRT_EXEC_UNIT_UNRECOVERABLE; a spaced retry usually succeeds."""
    import time
    from concourse import bass_utils
    last = None
    for k in range(attempts):
        try:
            return bass_utils.run_bass_kernel_spmd(
                nc, in_maps, core_ids=list(range(N_CORES)))
        except Exception as e:  # noqa: BLE001 - device transient
            last = e
            time.sleep(3.0 * (k + 1))
    raise last


def kernel(x_src, x_dst, edge_label_index, weight, bias):
    x_src = np.ascontiguousarray(np.asarray(x_src, dtype=np.float32))
    x_dst = np.ascontiguousarray(np.asarray(x_dst, dtype=np.float32))
    idx = np.asarray(edge_label_index)
    i0 = idx[0].astype(np.int64)
    i1 = idx[1].astype(np.int64)
    wgt = np.asarray(weight, dtype=np.float32)
    b = np.asarray(bias, dtype=np.float32)

    if "l1" not in _CACHE:
        _CACHE["l1"] = _build_launch1()
    if "l2" not in _CACHE:
        _CACHE["l2"] = _build_launch2()
    nc1, nc2 = _CACHE["l1"], _CACHE["l2"]

    # w staged on partitions (K = h), one column per side
    wv = np.zeros((128, 2), BF16)
    wv[:, 0] = wgt[0, :HIDDEN].astype(BF16)
    wv[:, 1] = wgt[0, HIDDEN:].astype(BF16)

    in_maps1 = []
    for c in range(N_CORES):
        in_maps1.append({
            "xs": _stage_x(x_src[c * NS:(c + 1) * NS]),
            "xd": _stage_x(x_dst[c * NS:(c + 1) * NS]),
            "wv": wv,
            "biasr": np.full((128, 1), b[0], np.float32),
        })
    res1 = _run_with_retry(nc1, in_maps1)
    s_tab, d_tab = _decode_scores(
        [res1.results[c]["sc"] for c in range(N_CORES)])

    # gather score halves per edge (host permutation only); a node's slot in
    # the concatenated table is (n // NS) * NPAD + (n % NS)
    v0 = s_tab[(i0 // NS) * NPAD + i0 % NS]
    v1 = d_tab[(i1 // NS) * NPAD + i1 % NS]
    in_maps2 = []
    for c in range(N_CORES):
        a = np.zeros((2, E_OUT), BF16)
        a[0, :PER] = v0[c * PER:(c + 1) * PER]
        a[1, :PER] = v1[c * PER:(c + 1) * PER]
        in_maps2.append({
            "a01": np.ascontiguousarray(
                a.reshape(2, 128, COLS).transpose(1, 0, 2)),
        })
    res2 = _run_with_retry(nc2, in_maps2)

    out = np.empty(N_EDGES, np.float32)
    for c in range(N_CORES):
        out[c * PER:(c + 1) * PER] = \
            res2.results[c]["o"].reshape(-1)[:PER].astype(np.float32)
    return out.reshape(N_EDGES, 1)
